# revision 50
# baseline (speedup 1.0000x reference)
"""FFTConv1d-with-threshold kernel for Trainium2, 8 NeuronCores.

Math: the reference (flat 16900-pt FFT -> prune coeffs with |Re|<0.01 ->
multiply by kernel FFT -> iFFT -> roll -> channel-sum -> slice) equals a
standard 3x3 pad-1 conv2d applied to (xp - delta), where delta is the
inverse FFT of the pruned (below-threshold) coefficients.

Per core (core = (batch b, out-channel half)):
  Cooley-Tukey 100x169 forward FFT as stationary-operand PE matmuls
  (stage-1 stationary = x0 channel-slices so outputs land b-major and
  stage-2 contracts b directly -- no transposes anywhere).  Threshold
  mask on DVE/Pool, inverse transform in bf16 (stage-1 stationary =
  masked-spectrum slices, again transpose-free), twiddles on DVE/Pool
  with stride-0 broadcast APs, delta subtracted per channel, flattened
  to image layout with overlapped partition-collapse DMAs, and the 3x3
  conv runs as 9 PSUM-accumulated fp32r matmuls over shift-duplicated
  quarters gathered SBUF-to-SBUF.
Forward Re path stays fp32 (threshold needs ~1e-3 absolute accuracy on
X values ~O(500); fp32r/bf16 measured too coarse on HW).  Everything
downstream of the mask only shapes delta (~1% of the signal) -> bf16.
"""

import numpy as np

import bass_rust
import concourse.bass as bass
import concourse.mybir as mybir
from concourse.bass_utils import run_bass_kernel_spmd
from concourse.tile import TileContext

F32 = mybir.dt.float32
F32R = mybir.dt.float32r
BF16 = mybir.dt.bfloat16

N1, N2, N = 100, 169, 16900
W130 = 130
B, C, O = 4, 32, 32
OH = O // 2
THRESH = 0.01
BBLK = [(0, 128), (128, 41)]   # b-axis split (169 = 128 + 41)


def _split_excess_waits(nc):
    # This walrus build accepts 1 sync-wait slot per instruction; Tile can
    # attach several. Move extras onto nofuse NOPs on the same engine.
    for f in nc.m.functions:
        for blk in f.blocks:
            insts = blk.instructions
            changed = False
            new_list = []
            for inst in insts:
                si = inst.sync_info
                if si is not None and len(si.on_wait) > 1:
                    waits = list(si.on_wait)
                    extra, keep = waits[:-1], waits[-1:]
                    for k, w in enumerate(extra):
                        new_list.append(bass_rust.InstNoOp(
                            name=f"{inst.name}-ws{k}",
                            engine=inst.engine,
                            ins=[], outs=[], bass_nofuse=True,
                            sync_info=bass_rust.SyncInfo(on_wait=[w], on_update=[]),
                        ))
                    inst.sync_info = bass_rust.SyncInfo(
                        on_wait=keep, on_update=list(si.on_update))
                    changed = True
                new_list.append(inst)
            if changed:
                blk.instructions = new_list


def _bcast(ap, n):
    # [p, x] -> [p, n, x] with stride-0 middle dim
    p, x = ap.shape
    return ap.rearrange("p (o x) -> p o x", o=1).broadcast_to((p, n, x))


def _build():
    nc = bass.Bass("TRN2")
    x0 = nc.dram_tensor("x0", [N1, C * N2], F32, kind="ExternalInput")
    wk = nc.dram_tensor("wk", [96, 3 * OH], BF16, kind="ExternalInput")
    bias_h = nc.dram_tensor("bias_h", [OH, 1], F32, kind="ExternalInput")
    cn = {}
    for name, shape in [
        ("c100cat", [N1, 200]),
        ("t1c_a", [128, N1]), ("t1s_a", [128, N1]),
        ("t1c_b", [41, N1]), ("t1s_b", [41, N1]),
        ("c169c_a", [128, 85]), ("c169c_b", [41, 85]),
        ("c169s_a", [128, 85]), ("c169s_b", [41, 85]),
        ("wgtx", [85, 400]),
        # below are loaded as bf16 via gpsimd cast DMAs
        ("c169c16_a", [128, 85]), ("c169c16_b", [41, 85]),
        ("c169sn16_a", [128, 85]), ("c169sn16_b", [41, 85]),
        ("invcat1", [85, 2 * N2]), ("invcat2", [85, 2 * N2]),
        ("t2c", [N1, N2]), ("t2s", [N1, N2]),
        ("cinvc", [N1, N1]), ("cinvsn", [N1, N1]),
    ]:
        dt = BF16 if name in ("c169c16_a", "c169c16_b", "c169sn16_a",
                              "c169sn16_b", "invcat1", "invcat2",
                              "t2c", "t2s", "cinvc", "cinvsn") else F32
        cn[name] = nc.dram_tensor(name, shape, dt, kind="ExternalInput")
    out_part = nc.dram_tensor("out_part", [OH, 128 * 128], BF16, kind="ExternalOutput")

    BF16_CONSTS = {"c169c16_a", "c169c16_b", "c169sn16_a", "c169sn16_b",
                   "invcat1", "invcat2", "t2c", "t2s", "cinvc", "cinvsn"}

    with TileContext(nc) as tc:
        with tc.tile_pool(name="const", bufs=1) as cst, \
             tc.tile_pool(name="big", bufs=1) as big:

            # ---- constants (critical-path order on sync; bf16 on gpsimd) ----
            ct = {}
            ct["c100cat"] = cst.tile([N1, 200], F32, tag="c100cat", name="c100cat")
            nc.sync.dma_start(out=ct["c100cat"][:], in_=cn["c100cat"][:])

            x0t = big.tile([N1, C * N2], F32, tag="x0")
            nc.sync.dma_start(out=x0t[:, 0:338], in_=x0[:, 0:338])
            for f in range(3):
                sl = bass.ds(338 + f * 1690, 1690)
                nc.sync.dma_start(out=x0t[:, sl], in_=x0[:, sl])
                if f == 0:
                    for name in ("t1c_a", "t1s_a", "t1c_b", "t1s_b"):
                        ct[name] = cst.tile(list(cn[name].shape), F32, tag=name, name=name)
                        nc.sync.dma_start(out=ct[name][:], in_=cn[name][:])
                if f == 1:
                    for name in ("c169c_a", "c169c_b", "c169s_a", "c169s_b",
                                 "wgtx"):
                        ct[name] = cst.tile(list(cn[name].shape), F32, tag=name, name=name)
                        nc.sync.dma_start(out=ct[name][:], in_=cn[name][:])
            for name in sorted(BF16_CONSTS):
                ct[name] = cst.tile(list(cn[name].shape), BF16, tag=name, name=name)
                nc.sync.dma_start(out=ct[name][:], in_=cn[name][:])
            wk_t = cst.tile([96, 3 * OH], BF16, tag="wk")
            nc.sync.dma_start(out=wk_t[:], in_=wk[:])
            bias_t = cst.tile([OH, 1], F32, tag="bias")
            nc.sync.dma_start(out=bias_t[:], in_=bias_h[:])

            xflat = big.tile([C, 16904], BF16, tag="xflat")

            # ---------- FFT + mask + inverse, pipelined over chunks ----------
            fftp_cm = [tc.tile_pool(name="g1sb", bufs=2),
                       tc.tile_pool(name="gtt", bufs=2),
                       tc.tile_pool(name="chunk", bufs=2),
                       tc.tile_pool(name="xtp", bufs=6),
                       tc.tile_pool(name="ps1a", bufs=1, space="PSUM"),
                       tc.tile_pool(name="ps1b", bufs=1, space="PSUM"),
                       tc.tile_pool(name="ps2", bufs=2, space="PSUM"),
                       tc.tile_pool(name="ps2i", bufs=1, space="PSUM"),
                       tc.tile_pool(name="psh", bufs=2, space="PSUM"),
                       tc.tile_pool(name="psd", bufs=1, space="PSUM")]
            g1p, gtp, chk, xtp, ps1a, ps1b, ps2, ps2i, psh, psd = \
                [cm.__enter__() for cm in fftp_cm]

            # variable-size megagroups (in 2-ch s1-groups): small first/last
            # megas shorten pipeline fill and drain.
            MEGA_GROUPS = [(0, 4), (4, 4), (8, 4), (12, 4)]
            CHUNKS = []          # f -> (mg, half)
            for _mg, (_g0, _ng) in enumerate(MEGA_GROUPS):
                for _h in range(_ng // 2):
                    CHUNKS.append((_mg, _h))
            mega = {}

            def fwd_s1_group(mg, sub):
                # 2 channels: stationary = x0 slices, rhs = [cos | -sin]
                # -> psum [bn, 400] -> Act drain to SBUF staging
                g0, ng = MEGA_GROUPS[mg]
                g = g0 + sub
                if sub == 0:
                    mega[mg] = (g1p.tile([128, 1600], F32, tag="g1a", name="g1a"),
                                g1p.tile([41, 1600], F32, tag="g1b", name="g1b"))
                pa = ps1a.tile([128, 400], F32, tag="s1a")
                pb = ps1b.tile([41, 400], F32, tag="s1b")
                for cl in range(2):
                    c = 2 * g + cl
                    for (b0, bn), pt in ((BBLK[0], pa), (BBLK[1], pb)):
                        lhsT = x0t[:, bass.ds(N2 * c + b0, bn)]
                        nc.tensor.matmul(pt[0:bn, bass.ts(cl, 200)],
                                         lhsT, ct["c100cat"][:],
                                         start=True, stop=True)
                g1a, g1b = mega[mg]
                nc.scalar.copy(out=g1a[:, bass.ts(sub, 400)], in_=pa[:])
                nc.scalar.copy(out=g1b[:, bass.ts(sub, 400)], in_=pb[:])

            def fwd_tw1_mega(mg):
                # twiddle1 on the mega's channels: Gt = G * exp(-2pi i r b / N)
                g0, ng = MEGA_GROUPS[mg]
                nch = 2 * ng
                g1a, g1b = mega[mg]
                out = {}
                for (b0, bn), g1 in ((BBLK[0], g1a), (BBLK[1], g1b)):
                    sfx = "a" if b0 == 0 else "b"
                    t1c = _bcast(ct["t1c_" + sfx][:], nch)
                    t1s = _bcast(ct["t1s_" + sfx][:], nch)
                    gv = g1[0:bn, 0:200 * nch].rearrange(
                        "p (c t r) -> p c t r", t=2, r=N1)
                    gre, gim = gv[:, :, 0, :], gv[:, :, 1, :]
                    gre_f = gtp.tile([128, 800], F32, tag="gre_" + sfx)
                    gim_f = gtp.tile([128, 800], F32, tag="gim_" + sfx)
                    gre16 = gtp.tile([128, 800], BF16, tag="gre16_" + sfx)
                    gim16 = gtp.tile([128, 800], BF16, tag="gim16_" + sfx)
                    rfv = gre_f[0:bn, 0:100 * nch].rearrange(
                        "p (c r) -> p c r", r=N1)
                    ifv = gim_f[0:bn, 0:100 * nch].rearrange(
                        "p (c r) -> p c r", r=N1)
                    mt = [chk.tile([128, 800], F32, tag=f"m{i}", name=f"m{i}",
                                   bufs=1 if i in (0, 1, 2) else 2)
                          for i in range(4)]
                    mv = [m[0:bn, 0:100 * nch].rearrange("p (c r) -> p c r", r=N1)
                          for m in mt]
                    # re = Gre*c + Gim*s ; im = Gim*c - Gre*s
                    nc.vector.tensor_mul(out=mv[0], in0=gre, in1=t1c)
                    nc.vector.tensor_mul(out=mv[1], in0=gim, in1=t1s)
                    nc.vector.tensor_add(out=rfv, in0=mv[0], in1=mv[1])
                    nc.gpsimd.tensor_mul(out=mv[2], in0=gim, in1=t1c)
                    nc.vector.tensor_mul(out=mv[3], in0=gre, in1=t1s)
                    nc.gpsimd.tensor_sub(out=ifv, in0=mv[2], in1=mv[3])
                    nc.scalar.copy(out=gre16[0:bn, 0:100 * nch],
                                   in_=gre_f[0:bn, 0:100 * nch])
                    nc.scalar.copy(out=gim16[0:bn, 0:100 * nch],
                                   in_=gim_f[0:bn, 0:100 * nch])
                    out[sfx] = (gre_f, gim_f, gre16, gim16)
                mega[mg] = out

            pending_back = []   # (f, half, hwre, hwim) inverse-stage-2 work

            def emit_back():
                # inverse stage 2 + subtract + flatten for one queued half
                f, hh, hwre, hwim = pending_back.pop(0)
                c0 = 4 * f + 2 * hh
                pd = psd.tile([N1, 2 * N2], F32, tag="dlt")
                nc.tensor.matmul(pd[:], ct["cinvc"][:], hwre[:],
                                 start=True, stop=False)
                nc.tensor.matmul(pd[:], ct["cinvsn"][:], hwim[:],
                                 start=False, stop=True)
                xt = xtp.tile([N1, 2 * N2], BF16, tag="xt")
                nc.vector.tensor_sub(out=xt[:],
                                     in0=x0t[:, bass.ds(N2 * c0, 2 * N2)],
                                     in1=pd[:])
                for cl in range(2):
                    nc.sync.dma_start(out=xflat[c0 + cl:c0 + cl + 1, 0:N],
                                      in_=xt[:, bass.ts(cl, N2)])

            def fwd_s2_mask_inv(f):
                # stage 2 (4 channels c=4f..4f+4), mask, inverse stage 1
                mg, half = CHUNKS[f]
                ga, gb = mega[mg]["a"], mega[mg]["b"]
                sl = bass.ts(half, 400)
                px = ps2.tile([85, 400], F32, tag="s2re")
                pxi = ps2i.tile([85, 400], F32, tag="s2im")
                nc.tensor.matmul(px[:], ct["c169c_a"][:], ga[0][:, sl],
                                 start=True, stop=False)
                nc.tensor.matmul(px[:], ct["c169c_b"][:], gb[0][0:41, sl],
                                 start=False, stop=False)
                nc.tensor.matmul(px[:], ct["c169s_a"][:], ga[1][:, sl],
                                 start=False, stop=False)
                nc.tensor.matmul(px[:], ct["c169s_b"][:], gb[1][0:41, sl],
                                 start=False, stop=True)
                nc.tensor.matmul(pxi[:], ct["c169c16_a"][:], ga[3][:, sl],
                                 start=True, stop=False)
                nc.tensor.matmul(pxi[:], ct["c169c16_b"][:], gb[3][0:41, sl],
                                 start=False, stop=False)
                nc.tensor.matmul(pxi[:], ct["c169sn16_a"][:], ga[2][:, sl],
                                 start=False, stop=False)
                nc.tensor.matmul(pxi[:], ct["c169sn16_b"][:], gb[2][0:41, sl],
                                 start=False, stop=True)
                # fill the mask-chain PE gap with queued inverse-stage-2 work
                while pending_back:
                    emit_back()
                # mask (all-DVE): pm = (|Xre| < t) * wgtx ; z = X * pm
                pm = chk.tile([85, 400], F32, tag="pm")
                nc.scalar.activation(pm[:], px[:],
                                     mybir.ActivationFunctionType.Abs)
                nc.vector.tensor_scalar(
                    out=pm[:], in0=pm[:], scalar1=THRESH, scalar2=None,
                    op0=mybir.AluOpType.is_lt)
                engm = nc.gpsimd if f < 5 else nc.vector
                engm.tensor_mul(out=pm[:], in0=pm[:], in1=ct["wgtx"][:])
                zr = chk.tile([85, 400], BF16, tag="zre")
                zi = chk.tile([85, 400], BF16, tag="zim")
                nc.vector.tensor_mul(out=zr[:], in0=px[:], in1=pm[:])
                nc.vector.tensor_mul(out=zi[:], in0=pxi[:], in1=pm[:])
                # inverse stage 1 (stationary = z-slices -> HT[k1, b] per ch)
                # + twiddle2, in 2-channel halves so tw2 starts early
                t2cv = _bcast(ct["t2c"][:], 2)
                t2sv = _bcast(ct["t2s"][:], 2)
                for hh in range(2):
                    htw = chk.tile([N1, 2 * 2 * N2], BF16, tag=f"htw{hh}",
                                   name=f"htw{hh}")
                    for cl in range(2):
                        ph = psh.tile([N1, 2 * N2], F32, tag="ht")
                        zr_s = zr[:, bass.ts(2 * hh + cl, N1)]
                        zi_s = zi[:, bass.ts(2 * hh + cl, N1)]
                        nc.tensor.matmul(ph[:], zr_s, ct["invcat1"][:],
                                         start=True, stop=False)
                        nc.tensor.matmul(ph[:], zi_s, ct["invcat2"][:],
                                         start=False, stop=True)
                        nc.scalar.copy(out=htw[:, bass.ds(2 * N2 * cl, 2 * N2)],
                                       in_=ph[:])
                    hv = htw[:].rearrange("p (c t x) -> p c t x", t=2, x=N2)
                    hre, him = hv[:, :, 0, :], hv[:, :, 1, :]
                    wt = [chk.tile([N1, 2 * N2], BF16, tag=f"w{i}",
                                   name=f"w{i}", bufs=2)
                          for i in range(4)]
                    wv = [w[:].rearrange("p (c x) -> p c x", x=N2) for w in wt]
                    hwre = chk.tile([N1, 2 * N2], BF16, tag=f"hwre{hh}",
                                    name=f"hwre{hh}")
                    hwim = chk.tile([N1, 2 * N2], BF16, tag=f"hwim{hh}",
                                    name=f"hwim{hh}")
                    hwre_v = hwre[:].rearrange("p (c x) -> p c x", x=N2)
                    hwim_v = hwim[:].rearrange("p (c x) -> p c x", x=N2)
                    eng2 = nc.gpsimd if f >= 5 else nc.vector
                    nc.vector.tensor_mul(out=wv[0], in0=hre, in1=t2cv)
                    eng2.tensor_mul(out=wv[1], in0=him, in1=t2sv)
                    nc.vector.tensor_sub(out=hwre_v, in0=wv[0], in1=wv[1])
                    eng2.tensor_mul(out=wv[2], in0=hre, in1=t2sv)
                    nc.vector.tensor_mul(out=wv[3], in0=him, in1=t2cv)
                    eng2.tensor_add(out=hwim_v, in0=wv[2], in1=wv[3])
                    pending_back.append((f, hh, hwre, hwim))

            # pipeline: keep PE ahead -- emit next mega's stage-1 before
            # this mega's stage-2 chain.
            for sub in range(MEGA_GROUPS[0][1]):
                fwd_s1_group(0, sub)
            ci = 0
            for mg in range(len(MEGA_GROUPS)):
                if mg + 1 < len(MEGA_GROUPS):
                    for sub in range(MEGA_GROUPS[mg + 1][1]):
                        fwd_s1_group(mg + 1, sub)
                fwd_tw1_mega(mg)
                for _h in range(MEGA_GROUPS[mg][1] // 2):
                    fwd_s2_mask_inv(ci)
                    ci += 1
            while pending_back:
                emit_back()

            for cm in reversed(fftp_cm):
                cm.__exit__(None, None, None)

            # ---------- conv 3x3 valid on 130x130 + bias ----------
            with tc.tile_pool(name="xq", bufs=2) as xqp, \
                 tc.tile_pool(name="ost", bufs=3) as ostp, \
                 tc.tile_pool(name="psc", bufs=3, space="PSUM") as psc:
                def gather_q(g):
                    n0 = 4160 * g
                    xtq = xqp.tile([96, 4420], BF16, tag="xtq")
                    for s in range(3):
                        nc.gpsimd.dma_start(
                            out=xtq[32 * s:32 * s + 16, 0:4420],
                            in_=xflat[0:16, n0 + s:n0 + s + 4420])
                        nc.sync.dma_start(
                            out=xtq[32 * s + 16:32 * s + 32, 0:4420],
                            in_=xflat[16:32, n0 + s:n0 + s + 4420])
                    return xtq

                xtq_cur = gather_q(0)
                for g in range(4):
                    xtq, xtq_cur = xtq_cur, None
                    if g < 3:
                        xtq_cur = gather_q(g + 1)   # prefetch before outs(g)
                    for lp2 in range(4):
                        ost = ostp.tile([OH, 1024], BF16, tag="ost")
                        ps_o = psc.tile([OH, 1024], F32, tag="conv")
                        for j in range(2):
                            lp = 2 * lp2 + j
                            for r in range(3):
                                off0 = (4 * lp + r) * W130
                                rhs = xtq[0:96, bass.ds(off0, 4 * W130)] \
                                    .rearrange("c (i w) -> c i w", w=W130)[:, :, 0:128]
                                lhsT = wk_t[:, bass.ts(r, OH)]
                                nc.tensor.matmul(
                                    ps_o[:, bass.ts(j, 512)]
                                    .rearrange("o (i t) -> o i t", t=128),
                                    lhsT, rhs,
                                    start=(r == 0), stop=(r == 2))
                        nc.scalar.activation(
                            ost[:], ps_o[:],
                            mybir.ActivationFunctionType.Identity,
                            bias=bias_t[:], scale=1.0)
                        nc.sync.dma_start(
                            out=out_part[:, bass.ts(4 * g + lp2, 1024)],
                            in_=ost[:])

    _split_excess_waits(nc)
    return nc


_NC_CACHE = {}


def _get_nc():
    if "nc" not in _NC_CACHE:
        _NC_CACHE["nc"] = _build()
    return _NC_CACHE["nc"]


def _consts():
    if "consts" in _NC_CACHE:
        return _NC_CACHE["consts"]
    r = np.arange(N1)
    q = np.arange(N2)
    a100 = 2 * np.pi * np.outer(r, r) / N1          # [a, r]
    a169 = 2 * np.pi * np.outer(q, q) / N2          # [b, q]
    t1 = 2 * np.pi * np.outer(q, r) / N             # [b, r]
    t2 = 2 * np.pi * np.outer(r, q) / N             # [k1, b]
    qh = np.arange(85)
    ainv = 2 * np.pi * np.outer(qh, q) / N2         # [q, b]

    cc = {}
    cc["c100cat"] = np.concatenate([np.cos(a100), -np.sin(a100)], axis=1)
    cc["t1c_a"] = np.cos(t1[0:128])
    cc["t1s_a"] = np.sin(t1[0:128])
    cc["t1c_b"] = np.cos(t1[128:169])
    cc["t1s_b"] = np.sin(t1[128:169])
    c169c = np.cos(a169)
    c169s = np.sin(a169)
    cc["c169c_a"] = c169c[0:128, 0:85]
    cc["c169c_b"] = c169c[128:169, 0:85]
    cc["c169s_a"] = c169s[0:128, 0:85]
    cc["c169s_b"] = c169s[128:169, 0:85]
    cc["c169c16_a"] = cc["c169c_a"]
    cc["c169c16_b"] = cc["c169c_b"]
    cc["c169sn16_a"] = -cc["c169s_a"]
    cc["c169sn16_b"] = -cc["c169s_b"]
    # conjugate-symmetry doubling weights for half-spectrum q in [0,85)
    wgt = np.full((85, N1), 2.0)
    wgt[0, 0] = 1.0
    wgt[84, 1:] = 1.0
    cc["wgtx"] = np.tile(wgt, (1, 4))
    invc = np.cos(ainv) / N2
    invs = np.sin(ainv) / N2
    cc["invcat1"] = np.concatenate([invc, invs], axis=1)
    cc["invcat2"] = np.concatenate([-invs, invc], axis=1)
    cc["t2c"] = np.cos(t2)
    cc["t2s"] = np.sin(t2)
    cc["cinvc"] = np.cos(a100) / N1                 # [k1, a]
    cc["cinvsn"] = -np.sin(a100) / N1
    import ml_dtypes
    BF16_CONSTS = {"c169c16_a", "c169c16_b", "c169sn16_a", "c169sn16_b",
                   "invcat1", "invcat2", "t2c", "t2s", "cinvc", "cinvsn"}
    cc = {k: np.ascontiguousarray(
        v, dtype=ml_dtypes.bfloat16 if k in BF16_CONSTS else np.float32)
        for k, v in cc.items()}
    _NC_CACHE["consts"] = cc
    return cc


def kernel(x, weight, bias):
    x = np.asarray(x, dtype=np.float32)
    weight = np.asarray(weight, dtype=np.float32)
    bias = np.asarray(bias, dtype=np.float32)
    nc = _get_nc()
    cc = _consts()

    xp = np.pad(x, ((0, 0), (0, 0), (1, 1), (1, 1)))          # (4,32,130,130)
    # [a, (c, b)] layout of the flat 16900 signal, per batch
    x0s = [np.ascontiguousarray(
        xp[b].reshape(C, N).reshape(C, N1, N2).transpose(1, 0, 2).reshape(N1, C * N2))
        for b in range(B)]

    in_maps = []
    for core in range(8):
        b, h = core // 2, core % 2
        o0 = h * OH
        wkm = np.empty((96, 3 * OH), dtype=np.float32)
        for s in range(3):
            for r in range(3):
                wkm[32 * s:32 * s + 32, r * OH:(r + 1) * OH] = weight[o0:o0 + OH, :, r, s].T
        import ml_dtypes as _mld
        m = {"x0": x0s[b], "wk": wkm.astype(_mld.bfloat16),
             "bias_h": np.ascontiguousarray(bias[o0:o0 + OH, None])}
        m.update(cc)
        in_maps.append(m)

    res = run_bass_kernel_spmd(nc, in_maps, core_ids=list(range(8)))

    out = np.empty((B, O, 128, 128), dtype=np.float32)
    for core in range(8):
        b, h = core // 2, core % 2
        out[b, h * OH:(h + 1) * OH] = np.asarray(
            res.results[core]["out_part"], dtype=np.float32).reshape(OH, 128, 128)
    return out
